# revision 1
# baseline (speedup 1.0000x reference)
"""v2: single-matmul + PE-transpose scheme, float32r x3.

Each core computes its A-row slab sim [1536, 12288] once (f32r hi/lo x3
matmuls). Direction A->B reduces rows on VectorE directly; direction B->A
is obtained by transposing each sim tile on TensorE (exact, fp32) and
reducing the transposed column strips; per-core partial column top-2s are
combined across cores on the host.
"""
import sys

sys.path.insert(0, '/opt/trn_rl_repo')

import numpy as np

CH = 512
N1 = 96 * 128
N2 = 96 * 128
N_CORES = 8
SLAB = N1 // N_CORES          # 1536
M_TILES = SLAB // 128         # 12
KT = CH // 128                # 4
CB = 1024                     # matmul block width (2 psum banks)
NCB = N2 // CB                # 12
RATIO = 0.95
EPS = 1e-8

_compiled = None
LAST_EXEC_NS = None
LAST_RESULTS = None


def _build():
    import concourse.bacc as bacc
    import concourse.tile as tile
    from concourse import mybir

    nc = bacc.Bacc("TRN2", target_bir_lowering=False, debug=False,
                   num_devices=N_CORES)

    lhsT_d = [nc.dram_tensor(f"lhsT{p}", [CH, SLAB], mybir.dt.float32r,
                             kind="ExternalInput") for p in "hl"]
    rhs_d = [nc.dram_tensor(f"rhs{p}", [CH, N2], mybir.dt.float32r,
                            kind="ExternalInput") for p in "hl"]
    vals1_d = nc.dram_tensor("vals1", [M_TILES, 128, NCB, 8],
                             mybir.dt.float32, kind="ExternalOutput")
    idxs1_d = nc.dram_tensor("idxs1", [M_TILES, 128, NCB, 8],
                             mybir.dt.uint32, kind="ExternalOutput")
    vals2_d = nc.dram_tensor("vals2", [NCB, 2, 8, 128, 8],
                             mybir.dt.float32, kind="ExternalOutput")
    idxs2_d = nc.dram_tensor("idxs2", [NCB, 2, 8, 128, 8],
                             mybir.dt.uint32, kind="ExternalOutput")

    with tile.TileContext(nc) as tc:
        with tc.tile_pool(name="lhs", bufs=1) as lhs_pool, \
             tc.tile_pool(name="rhs", bufs=2) as rhs_pool, \
             tc.tile_pool(name="sim", bufs=3) as sim_pool, \
             tc.tile_pool(name="strips", bufs=1) as strip_pool, \
             tc.tile_pool(name="psmm", bufs=2, space="PSUM") as psmm_pool, \
             tc.tile_pool(name="pstr", bufs=3, space="PSUM") as pstr_pool, \
             tc.tile_pool(name="stats", bufs=1) as stats_pool:
            lh = lhs_pool.tile([128, KT, SLAB], mybir.dt.float32r, tag="lh")
            ll = lhs_pool.tile([128, KT, SLAB], mybir.dt.float32r, tag="ll")
            for t, src in ((lh, lhsT_d[0]), (ll, lhsT_d[1])):
                nc.sync.dma_start(
                    out=t[:],
                    in_=src.ap().rearrange("(kt p) m -> p kt m", p=128))
            ident_d = nc.inline_tensor(np.eye(128, dtype=np.float32),
                                       name="ident")
            ident = lhs_pool.tile([128, 128], mybir.dt.float32, tag="ident")
            nc.sync.dma_start(out=ident[:], in_=ident_d.ap())

            sv1 = stats_pool.tile([128, M_TILES, NCB, 8], mybir.dt.float32,
                                  tag="sv1")
            si1 = stats_pool.tile([128, M_TILES, NCB, 8], mybir.dt.uint32,
                                  tag="si1")
            sv2 = stats_pool.tile([128, NCB, 2, 8, 8], mybir.dt.float32,
                                  tag="sv2")
            si2 = stats_pool.tile([128, NCB, 2, 8, 8], mybir.dt.uint32,
                                  tag="si2")

            for cb in range(NCB):
                rh = rhs_pool.tile([128, KT, CB], mybir.dt.float32r, tag="rh")
                rl = rhs_pool.tile([128, KT, CB], mybir.dt.float32r, tag="rl")
                for t, src in ((rh, rhs_d[0]), (rl, rhs_d[1])):
                    nc.sync.dma_start(
                        out=t[:],
                        in_=src.ap()[:, cb * CB:(cb + 1) * CB]
                        .rearrange("(kt p) n -> p kt n", p=128))

                # strips[:, b, :] holds cols cb*CB + b*128 .. +128 (partition
                # = col within block), rows = one half of the slab's A-rows;
                # halves are reduced separately (combined on host) so the
                # strip buffer can double-buffer across cb boundaries.
                for mh in range(2):
                  strips = strip_pool.tile([128, 8, SLAB // 2],
                                           mybir.dt.float32, tag="strips")
                  for m in range(mh * 6, mh * 6 + 6):
                    ps = psmm_pool.tile([128, CB], mybir.dt.float32, tag="ps")
                    msl = slice(m * 128, (m + 1) * 128)
                    lsl = slice((m - mh * 6) * 128, (m - mh * 6 + 1) * 128)
                    for k in range(KT):
                        for i, (lt, rt) in enumerate(
                                ((lh, rh), (lh, rl), (ll, rh))):
                            for c in range(CB // 512):
                                nc.tensor.matmul(
                                    out=ps[:, c * 512:(c + 1) * 512],
                                    lhsT=lt[:, k, msl],
                                    rhs=rt[:, k, c * 512:(c + 1) * 512],
                                    start=(k == 0 and i == 0),
                                    stop=(k == KT - 1 and i == 2))
                    s = sim_pool.tile([128, CB], mybir.dt.float32, tag="s")
                    nc.scalar.copy(s[:], ps[:])
                    # direction 1: rows are A points
                    nc.vector.max(sv1[:, m, cb], s[:])
                    nc.vector.max_index(si1[:, m, cb], sv1[:, m, cb], s[:])
                    # transpose the 8 [128,128] blocks, 4 per psum tile
                    for hb in range(2):
                        pt = pstr_pool.tile([128, 4, 128], mybir.dt.float32,
                                            tag="pt")
                        for j in range(4):
                            b = hb * 4 + j
                            nc.tensor.transpose(
                                pt[:, j], s[:, b * 128:(b + 1) * 128],
                                ident[:])
                        nc.scalar.copy(
                            strips[:, hb * 4:(hb + 1) * 4, lsl], pt[:])

                  for b in range(8):
                    nc.vector.max(sv2[:, cb, mh, b], strips[:, b])
                    nc.vector.max_index(si2[:, cb, mh, b], sv2[:, cb, mh, b],
                                        strips[:, b])

            nc.sync.dma_start(
                out=vals1_d.ap().rearrange("m p c e -> p m c e"), in_=sv1[:])
            nc.sync.dma_start(
                out=idxs1_d.ap().rearrange("m p c e -> p m c e"), in_=si1[:])
            nc.sync.dma_start(
                out=vals2_d.ap().rearrange("c h b p e -> p c h b e"),
                in_=sv2[:])
            nc.sync.dma_start(
                out=idxs2_d.ap().rearrange("c h b p e -> p c h b e"),
                in_=si2[:])

    nc.compile()
    return nc


def _get_compiled():
    global _compiled
    if _compiled is None:
        _compiled = _build()
    return _compiled


def _split_hi_lo(x):
    u = x.view(np.uint32)
    r = ((u + np.uint32(1 << 11)) & np.uint32(0xFFFFF000)).view(np.float32)
    return r, (x - r).astype(np.float32)


def _normalize(fmap):
    d = fmap.reshape(CH, -1).astype(np.float32)
    nrm = np.sqrt(np.sum(np.square(d), axis=0, keepdims=True,
                         dtype=np.float32))
    return (d / nrm).astype(np.float32)


def _combine(vals, idxs):
    """vals/idxs: [R, C, 8] chunk top-8s with idxs already global.
    Returns per-row top1 val, top1 idx, top2 val across all chunks."""
    c1 = vals[:, :, 0]
    c2 = vals[:, :, 1]
    j = np.argmax(c1, axis=1)
    r = np.arange(c1.shape[0])
    m1 = c1[r, j]
    i1 = idxs[r, j, 0].astype(np.int64)
    c1m = c1.copy()
    c1m[r, j] = -np.inf
    s = c1m.max(axis=1)
    m2 = np.maximum(s, c2[r, j])
    return m1, i1, m2


def _install_trace_shim():
    import types

    try:
        import antenv.axon_hooks  # noqa: F401
    except ImportError:
        from trn_agent_boot.trn_boot import _ntff_profile_via_ctypes
        hook = _ntff_profile_via_ctypes('/opt/axon/libaxon_pjrt.so')
        mod = types.ModuleType('antenv.axon_hooks')
        mod.get_axon_ntff_profile_hook = lambda: hook
        mod.set_axon_ntff_profile_hook = lambda h: None
        sys.modules['antenv.axon_hooks'] = mod
    import concourse.bass_utils as bu
    bu.upload_artifacts = lambda tmpdir: tmpdir


def kernel(map_A, map_B):
    import os

    from concourse.bass_utils import run_bass_kernel_spmd

    global LAST_EXEC_NS, LAST_RESULTS
    trace = bool(int(os.environ.get("KERNEL_TRACE", "0")))
    if trace:
        _install_trace_shim()
    nc = _get_compiled()

    nA = _normalize(np.asarray(map_A))
    nB = _normalize(np.asarray(map_B))
    nAh, nAl = _split_hi_lo(nA)
    nBh, nBl = _split_hi_lo(nB)

    in_maps = []
    for c in range(N_CORES):
        sl = slice(c * SLAB, (c + 1) * SLAB)
        in_maps.append({
            "lhsTh": np.ascontiguousarray(nAh[:, sl]),
            "lhsTl": np.ascontiguousarray(nAl[:, sl]),
            "rhsh": nBh,
            "rhsl": nBl,
        })

    res = run_bass_kernel_spmd(nc, in_maps, core_ids=list(range(N_CORES)),
                               trace=trace)
    LAST_EXEC_NS = res.exec_time_ns
    LAST_RESULTS = res

    # direction 1: concatenate row slabs; chunk idx offset = cb*CB
    mv, mi, ms = [], [], []
    off = (np.arange(NCB, dtype=np.int64) * CB)[None, :, None]
    for c in range(N_CORES):
        v = res.results[c]["vals1"].reshape(SLAB, NCB, 8)
        ix = res.results[c]["idxs1"].reshape(SLAB, NCB, 8).astype(np.int64)
        a, b, e = _combine(v, ix + off)
        mv.append(a)
        mi.append(b)
        ms.append(e)
    m1_12 = np.concatenate(mv)
    nn12 = np.concatenate(mi)
    m2_12 = np.concatenate(ms)

    # direction 2: per-core partial top-8 over its slab rows; combine cores
    # [NCB, 2, 8, 128, 8] -> per col (cb*CB + b*128 + p): 2 half-chunks
    v2 = np.stack([res.results[c]["vals2"].transpose(0, 2, 3, 1, 4)
                   .reshape(N2, 2, 8)
                   for c in range(N_CORES)], axis=1).reshape(N2, 2 * N_CORES, 8)
    half = (np.arange(2, dtype=np.int64) * (SLAB // 2))[None, :, None]
    i2 = np.stack([res.results[c]["idxs2"].transpose(0, 2, 3, 1, 4)
                   .reshape(N2, 2, 8).astype(np.int64) + half + c * SLAB
                   for c in range(N_CORES)], axis=1).reshape(N2, 2 * N_CORES, 8)
    m1_21, nn21, m2_21 = _combine(v2, i2)

    match_sim = m1_12
    ratios12 = (2.0 - 2.0 * m1_12) / ((2.0 - 2.0 * m2_12) + EPS)
    ratios21 = (2.0 - 2.0 * m1_21) / ((2.0 - 2.0 * m2_21) + EPS)

    ids1 = np.arange(N1)
    mask = ((ids1 == nn21[nn12]) & (ratios12 <= RATIO)
            & (ratios21[nn12] <= RATIO))
    masked_sim = np.where(mask, match_sim, 0.0).astype(np.float32)
    return masked_sim, nn12.astype(np.int32), mask



# revision 2
# speedup vs baseline: 2.4835x; 2.4835x over previous
"""v3: fp8 DoubleRow matmul + fp16 fold/pack candidate generation.

Each core computes its A-row slab sim [1536, 12288] once with fp8e4m3
DoubleRow matmuls (inputs scaled x32, so PSUM holds 1024*sim). The
Activation engine copies PSUM->SBUF as q = sim + 3 in fp16 (all values
in [2,4) => uniform 2^-10 grid, low mantissa bits zero). Direction A->B
folds q across the 12 column chunks into a per-lane running max (DVE
scalar_tensor_tensor, 4x mode); direction B->A transposes q tiles on
the PE (fp16) and folds across the 12 row blocks (2x mode). A final
top-8 per 128/1024 lanes runs on packed fp32 values q + lane*2^-22, so
the lane index is recovered exactly from the low mantissa bits without
a max_index pass. The host decodes lane winners, expands each lane to
its 12 fold members, and re-ranks candidates with exact fp32 dots to
produce the true top-2 per row/column, then applies the ratio and
mutual-NN tests.
"""
import sys

sys.path.insert(0, '/opt/trn_rl_repo')

import numpy as np

CH = 512
N1 = 96 * 128
N2 = 96 * 128
N_CORES = 8
SLAB = N1 // N_CORES          # 1536
MT = SLAB // 128              # 12 row tiles per core
CB = 1024                     # column chunk
NCB = N2 // CB                # 12
RATIO = 0.95
EPS = 1e-8
K1 = 5                        # lanes kept per row (dir 1)
K2 = 5                        # lane entries kept per column (dir 2)

_compiled = None
LAST_EXEC_NS = None
LAST_RESULTS = None


def _build():
    import concourse.bacc as bacc
    import concourse.tile as tile
    from concourse import mybir

    nc = bacc.Bacc("TRN2", target_bir_lowering=False, debug=False,
                   num_devices=N_CORES)

    lhs_d = nc.dram_tensor("lhs8", [CH, SLAB], mybir.dt.float8e4,
                           kind="ExternalInput")
    rhs_d = nc.dram_tensor("rhs8", [CH, N2], mybir.dt.float8e4,
                           kind="ExternalInput")
    o1_d = nc.dram_tensor("o1", [MT, 128, 8], mybir.dt.float32,
                          kind="ExternalOutput")
    o2_d = nc.dram_tensor("o2", [NCB, 8, 128, 8], mybir.dt.float32,
                          kind="ExternalOutput")

    lane1_np = np.broadcast_to(
        (np.arange(CB, dtype=np.float64) * 2.0**-22).astype(np.float32),
        (128, CB)).copy()
    lane2_np = np.broadcast_to(
        (np.arange(128, dtype=np.float64) * 2.0**-22).astype(np.float32),
        (128, 8, 128)).copy()

    with tile.TileContext(nc) as tc:
        with tc.tile_pool(name="wts", bufs=1) as wts_pool, \
             tc.tile_pool(name="rhs", bufs=2) as rhs_pool, \
             tc.tile_pool(name="q", bufs=3) as q_pool, \
             tc.tile_pool(name="acc", bufs=1) as acc_pool, \
             tc.tile_pool(name="acc2", bufs=2) as acc2_pool, \
             tc.tile_pool(name="pk", bufs=2) as pk_pool, \
             tc.tile_pool(name="out", bufs=1) as out_pool, \
             tc.tile_pool(name="psmm", bufs=2, space="PSUM") as psmm_pool, \
             tc.tile_pool(name="pstr", bufs=2, space="PSUM") as pstr_pool:
            lh = wts_pool.tile([128, 4, SLAB], mybir.dt.float8e4, tag="lh")
            nc.sync.dma_start(
                out=lh[:],
                in_=lhs_d.ap().rearrange("(kt p) m -> p kt m", p=128))
            ident_d = nc.inline_tensor(np.eye(128, dtype=np.float16),
                                       name="ident")
            ident = wts_pool.tile([128, 128], mybir.dt.float16, tag="ident")
            nc.sync.dma_start(out=ident[:], in_=ident_d.ap())
            lane1_d = nc.inline_tensor(lane1_np, name="lane1")
            lane1 = wts_pool.tile([128, CB], mybir.dt.float32, tag="lane1")
            nc.sync.dma_start(out=lane1[:], in_=lane1_d.ap())
            lane2_d = nc.inline_tensor(lane2_np, name="lane2")
            lane2 = wts_pool.tile([128, 8, 128], mybir.dt.float32,
                                  tag="lane2")
            nc.sync.dma_start(out=lane2[:], in_=lane2_d.ap())

            acc1 = acc_pool.tile([128, MT, CB], mybir.dt.float16, tag="acc1")
            nc.gpsimd.memset(acc1[:], 0.0)
            o1s = out_pool.tile([128, MT, 8], mybir.dt.float32, tag="o1s")
            o2s = out_pool.tile([128, NCB, 8, 8], mybir.dt.float32,
                                tag="o2s")

            for cb in range(NCB):
                rh = rhs_pool.tile([128, 4, CB], mybir.dt.float8e4, tag="rh")
                nc.sync.dma_start(
                    out=rh[:],
                    in_=rhs_d.ap()[:, cb * CB:(cb + 1) * CB]
                    .rearrange("(kt p) n -> p kt n", p=128))

                acc2 = acc2_pool.tile([128, 8, 128], mybir.dt.float16,
                                      tag="acc2")
                nc.gpsimd.memset(acc2[:], 0.0)

                for m in range(MT):
                    ps = psmm_pool.tile([128, CB], mybir.dt.float32,
                                        tag="ps")
                    msl = slice(m * 128, (m + 1) * 128)
                    for c in range(CB // 512):
                        csl = slice(c * 512, (c + 1) * 512)
                        for kp in range(2):
                            nc.tensor.matmul(
                                out=ps[:, csl],
                                lhsT=lh[:, 2 * kp:2 * kp + 2, msl],
                                rhs=rh[:, 2 * kp:2 * kp + 2, csl],
                                start=(kp == 0),
                                stop=(kp == 1),
                                perf_mode=mybir.MatmulPerfMode.DoubleRow)
                    # q = sim + 3 in [2, 4): fp16 => absolute 2^-10 grid
                    q = q_pool.tile([128, CB], mybir.dt.float16, tag="q")
                    nc.scalar.activation(
                        out=q[:], in_=ps[:],
                        func=mybir.ActivationFunctionType.Copy,
                        bias=3.0, scale=1.0 / 1024.0)
                    # dir 1: fold this chunk into the row-lane accumulator
                    nc.vector.scalar_tensor_tensor(
                        out=acc1[:, m], in0=acc1[:, m], scalar=1.0,
                        in1=q[:], op0=mybir.AluOpType.mult,
                        op1=mybir.AluOpType.max)
                    # dir 2: transpose the 8 [128,128] blocks and fold
                    # across row blocks
                    pt = pstr_pool.tile([128, 8, 128], mybir.dt.float16,
                                        tag="pt")
                    for b in range(8):
                        nc.tensor.transpose(
                            pt[:, b], q[:, b * 128:(b + 1) * 128], ident[:])
                    nc.vector.scalar_tensor_tensor(
                        out=acc2[:], in0=acc2[:], scalar=1.0,
                        in1=pt[:], op0=mybir.AluOpType.mult,
                        op1=mybir.AluOpType.max)

                pk2 = pk_pool.tile([128, 8, 128], mybir.dt.float32,
                                   tag="pk2")
                nc.vector.scalar_tensor_tensor(
                    out=pk2[:], in0=acc2[:], scalar=1.0, in1=lane2[:],
                    op0=mybir.AluOpType.mult, op1=mybir.AluOpType.add)
                for b in range(8):
                    nc.vector.max(o2s[:, cb, b], pk2[:, b])

            for m in range(MT):
                pk1 = pk_pool.tile([128, CB], mybir.dt.float32, tag="pk1")
                nc.vector.scalar_tensor_tensor(
                    out=pk1[:], in0=acc1[:, m], scalar=1.0, in1=lane1[:],
                    op0=mybir.AluOpType.mult, op1=mybir.AluOpType.add)
                nc.vector.max(o1s[:, m], pk1[:])

            nc.sync.dma_start(
                out=o1_d.ap().rearrange("m p e -> p m e"), in_=o1s[:])
            nc.sync.dma_start(
                out=o2_d.ap().rearrange("c b p e -> p c b e"), in_=o2s[:])

    nc.compile()
    return nc


def _get_compiled():
    global _compiled
    if _compiled is None:
        _compiled = _build()
    return _compiled


def _normalize(fmap):
    d = fmap.reshape(CH, -1).astype(np.float32)
    nrm = np.sqrt(np.sum(np.square(d), axis=0, keepdims=True,
                         dtype=np.float32))
    return (d / nrm).astype(np.float32)


def _install_trace_shim():
    import types

    try:
        import antenv.axon_hooks  # noqa: F401
    except ImportError:
        from trn_agent_boot.trn_boot import _ntff_profile_via_ctypes
        hook = _ntff_profile_via_ctypes('/opt/axon/libaxon_pjrt.so')
        mod = types.ModuleType('antenv.axon_hooks')
        mod.get_axon_ntff_profile_hook = lambda: hook
        mod.set_axon_ntff_profile_hook = lambda h: None
        sys.modules['antenv.axon_hooks'] = mod
    import concourse.bass_utils as bu
    bu.upload_artifacts = lambda tmpdir: tmpdir


def _rerank(dq, dv, cand):
    """Exact top-2 per point. dq [CH, P] query descs, dv [CH, NV] value
    descs, cand [P, K] candidate indices (ascending per row). Returns
    (m1, i1, m2): best val, best idx (lowest on ties), second val."""
    P, K = cand.shape
    m1 = np.empty(P, np.float32)
    m2 = np.empty(P, np.float32)
    i1 = np.empty(P, np.int64)
    dqT = dq.T
    dvT = dv.T
    step = 2048
    for s in range(0, P, step):
        e = min(s + step, P)
        g = dvT[cand[s:e]]                       # [n, K, CH]
        sims = np.einsum('nkc,nc->nk', g, dqT[s:e],
                         optimize=True).astype(np.float32)
        j = np.argmax(sims, axis=1)
        r = np.arange(e - s)
        m1[s:e] = sims[r, j]
        i1[s:e] = cand[s:e][r, j]
        sims[r, j] = -np.inf
        m2[s:e] = sims.max(axis=1)
    return m1, i1, m2


def kernel(map_A, map_B):
    import os

    from concourse.bass_utils import run_bass_kernel_spmd
    from concourse import mybir

    global LAST_EXEC_NS, LAST_RESULTS
    trace = bool(int(os.environ.get("KERNEL_TRACE", "0")))
    if trace:
        _install_trace_shim()
    nc = _get_compiled()

    fp8 = np.dtype(mybir.dt.np(mybir.dt.float8e4))
    nA = _normalize(np.asarray(map_A))
    nB = _normalize(np.asarray(map_B))
    a8 = (nA * 32.0).astype(fp8)
    b8 = (nB * 32.0).astype(fp8)

    in_maps = []
    for c in range(N_CORES):
        sl = slice(c * SLAB, (c + 1) * SLAB)
        in_maps.append({
            "lhs8": np.ascontiguousarray(a8[:, sl]),
            "rhs8": b8,
        })

    res = run_bass_kernel_spmd(nc, in_maps, core_ids=list(range(N_CORES)),
                               trace=trace)
    LAST_EXEC_NS = res.exec_time_ns
    LAST_RESULTS = res

    # ---- direction 1: per-row top lanes -> candidate columns ----
    # o1 [MT, 128, 8] packed; row = core*SLAB + m*128 + p
    p1 = np.stack([res.results[c]["o1"] for c in range(N_CORES)])
    p1 = p1.transpose(0, 1, 2, 3).reshape(N_CORES, MT * 128, 8)
    p1 = p1.reshape(N1, 8)                        # sorted desc per row
    lanes1 = (p1.view(np.uint32) & 0x3FF).astype(np.int64)[:, :K1]
    cols = (lanes1[:, :, None]
            + (np.arange(NCB, dtype=np.int64) * CB)[None, None, :])
    cols = np.sort(cols.reshape(N1, K1 * NCB), axis=1)
    m1_12, nn12, m2_12 = _rerank(nA, nB, cols)

    # ---- direction 2: per-column top lane entries -> candidate rows ----
    # o2 [NCB, 8, 128, 8]; col = cb*CB + b*128 + p; lane = row in block
    p2 = np.stack([res.results[c]["o2"] for c in range(N_CORES)], axis=0)
    p2 = p2.transpose(1, 2, 3, 0, 4).reshape(N2, N_CORES, 8)
    lanes2 = (p2.view(np.uint32) & 0x3FF).astype(np.int64)
    base = (np.arange(N_CORES, dtype=np.int64) * SLAB)[None, :, None]
    rowbase = (lanes2 + base).reshape(N2, N_CORES * 8)
    vals2 = p2.reshape(N2, N_CORES * 8)
    order = np.argsort(-vals2, axis=1)[:, :K2]
    rb = np.take_along_axis(rowbase, order, axis=1)
    rows = (rb[:, :, None]
            + (np.arange(MT, dtype=np.int64) * 128)[None, None, :])
    rows = np.sort(rows.reshape(N2, K2 * MT), axis=1)
    m1_21, nn21, m2_21 = _rerank(nB, nA, rows)

    match_sim = m1_12
    ratios12 = (2.0 - 2.0 * m1_12) / ((2.0 - 2.0 * m2_12) + EPS)
    ratios21 = (2.0 - 2.0 * m1_21) / ((2.0 - 2.0 * m2_21) + EPS)

    ids1 = np.arange(N1)
    mask = ((ids1 == nn21[nn12]) & (ratios12 <= RATIO)
            & (ratios21[nn12] <= RATIO))
    masked_sim = np.where(mask, match_sim, 0.0).astype(np.float32)
    return masked_sim, nn12.astype(np.int32), mask


# revision 4
# speedup vs baseline: 3.8768x; 1.5610x over previous
"""v3: fp8 DoubleRow matmul + fp16 fold/pack candidate generation.

Each core computes its A-row slab sim [1536, 12288] once with fp8e4m3
DoubleRow matmuls (inputs scaled x32, so PSUM holds 1024*sim). The
Activation engine copies PSUM->SBUF as q = sim + 3 in fp16 (all values
in [2,4) => uniform 2^-10 grid, low mantissa bits zero). Direction A->B
folds q across the 12 column chunks into a per-lane running max (DVE
scalar_tensor_tensor, 4x mode); direction B->A transposes q tiles on
the PE (fp16) and folds across the 12 row blocks (2x mode). A final
top-8 per 128/1024 lanes runs on packed fp32 values q + lane*2^-22, so
the lane index is recovered exactly from the low mantissa bits without
a max_index pass. The host decodes lane winners, expands each lane to
its 12 fold members, and re-ranks candidates with exact fp32 dots to
produce the true top-2 per row/column, then applies the ratio and
mutual-NN tests.
"""
import sys

sys.path.insert(0, '/opt/trn_rl_repo')

import numpy as np

CH = 512
N1 = 96 * 128
N2 = 96 * 128
N_CORES = 8
SLAB = N1 // N_CORES          # 1536
MT = SLAB // 128              # 12 row tiles per core
CB = 1024                     # column chunk
NCB = N2 // CB                # 12
RATIO = 0.95
EPS = 1e-8
K1 = 5                        # lanes kept per row (dir 1)
K2 = 5                        # lane entries kept per column (dir 2)

_compiled = None
LAST_EXEC_NS = None
LAST_RESULTS = None


def _build():
    import concourse.bacc as bacc
    import concourse.tile as tile
    from concourse import mybir

    nc = bacc.Bacc("TRN2", target_bir_lowering=False, debug=False,
                   num_devices=N_CORES)

    lhs_d = nc.dram_tensor("lhs8", [CH, SLAB], mybir.dt.float8e4,
                           kind="ExternalInput")
    rhs_d = nc.dram_tensor("rhs8", [CH, N2], mybir.dt.float8e4,
                           kind="ExternalInput")
    o1_d = nc.dram_tensor("o1", [MT, 128, 8], mybir.dt.float32,
                          kind="ExternalOutput")
    o2_d = nc.dram_tensor("o2", [NCB, 8, 128, 8], mybir.dt.float32,
                          kind="ExternalOutput")

    lane1_np = np.broadcast_to(
        (np.arange(CB, dtype=np.float64) * 2.0**-22).astype(np.float32),
        (128, CB)).copy()
    lane2_np = np.broadcast_to(
        (np.arange(128, dtype=np.float64) * 2.0**-22).astype(np.float32),
        (128, 8, 128)).copy()

    with tile.TileContext(nc) as tc:
        with tc.tile_pool(name="wts", bufs=1) as wts_pool, \
             tc.tile_pool(name="rhs", bufs=2) as rhs_pool, \
             tc.tile_pool(name="q", bufs=3) as q_pool, \
             tc.tile_pool(name="acc", bufs=1) as acc_pool, \
             tc.tile_pool(name="acc2", bufs=2) as acc2_pool, \
             tc.tile_pool(name="pk", bufs=2) as pk_pool, \
             tc.tile_pool(name="out", bufs=1) as out_pool, \
             tc.tile_pool(name="psmm", bufs=2, space="PSUM") as psmm_pool, \
             tc.tile_pool(name="pstr", bufs=2, space="PSUM") as pstr_pool:
            lh = wts_pool.tile([128, 4, SLAB], mybir.dt.float8e4, tag="lh")
            nc.sync.dma_start(
                out=lh[:],
                in_=lhs_d.ap().rearrange("(kt p) m -> p kt m", p=128))
            ident_d = nc.inline_tensor(np.eye(128, dtype=np.float16),
                                       name="ident")
            ident = wts_pool.tile([128, 128], mybir.dt.float16, tag="ident")
            nc.sync.dma_start(out=ident[:], in_=ident_d.ap())
            lane1_d = nc.inline_tensor(lane1_np, name="lane1")
            lane1 = wts_pool.tile([128, CB], mybir.dt.float32, tag="lane1")
            nc.sync.dma_start(out=lane1[:], in_=lane1_d.ap())
            lane2_d = nc.inline_tensor(lane2_np, name="lane2")
            lane2 = wts_pool.tile([128, 8, 128], mybir.dt.float32,
                                  tag="lane2")
            nc.sync.dma_start(out=lane2[:], in_=lane2_d.ap())

            acc1 = acc_pool.tile([128, MT, CB], mybir.dt.float16, tag="acc1")
            nc.gpsimd.memset(acc1[:], 0.0)
            o1s = out_pool.tile([128, MT, 8], mybir.dt.float32, tag="o1s")
            o2s = out_pool.tile([128, NCB, 8, 8], mybir.dt.float32,
                                tag="o2s")

            for cb in range(NCB):
                rh = rhs_pool.tile([128, 4, CB], mybir.dt.float8e4, tag="rh")
                nc.sync.dma_start(
                    out=rh[:],
                    in_=rhs_d.ap()[:, cb * CB:(cb + 1) * CB]
                    .rearrange("(kt p) n -> p kt n", p=128))

                acc2 = acc2_pool.tile([128, 8, 128], mybir.dt.float16,
                                      tag="acc2")
                nc.gpsimd.memset(acc2[:], 0.0)

                for m in range(MT):
                    ps = psmm_pool.tile([128, CB], mybir.dt.float32,
                                        tag="ps")
                    msl = slice(m * 128, (m + 1) * 128)
                    for c in range(CB // 512):
                        csl = slice(c * 512, (c + 1) * 512)
                        for kp in range(2):
                            nc.tensor.matmul(
                                out=ps[:, csl],
                                lhsT=lh[:, 2 * kp:2 * kp + 2, msl],
                                rhs=rh[:, 2 * kp:2 * kp + 2, csl],
                                start=(kp == 0),
                                stop=(kp == 1),
                                perf_mode=mybir.MatmulPerfMode.DoubleRow)
                    # q = sim + 3 in [2, 4): fp16 => absolute 2^-10 grid
                    q = q_pool.tile([128, CB], mybir.dt.float16, tag="q")
                    nc.scalar.activation(
                        out=q[:], in_=ps[:],
                        func=mybir.ActivationFunctionType.Copy,
                        bias=3.0, scale=1.0 / 1024.0)
                    # dir 1: fold this chunk into the row-lane accumulator
                    nc.vector.tensor_max(acc1[:, m], acc1[:, m], q[:])
                    # dir 2: transpose the 8 [128,128] blocks and fold
                    # across row blocks
                    pt = pstr_pool.tile([128, 8, 128], mybir.dt.float16,
                                        tag="pt")
                    for b in range(8):
                        nc.tensor.transpose(
                            pt[:, b], q[:, b * 128:(b + 1) * 128], ident[:])
                    nc.vector.tensor_max(acc2[:], acc2[:], pt[:])

                pk2 = pk_pool.tile([128, 8, 128], mybir.dt.float32,
                                   tag="pk2")
                nc.gpsimd.tensor_add(pk2[:], acc2[:], lane2[:])
                for b in range(8):
                    nc.vector.max(o2s[:, cb, b], pk2[:, b])

            for m in range(MT):
                pk1 = pk_pool.tile([128, CB], mybir.dt.float32, tag="pk1")
                nc.gpsimd.tensor_add(pk1[:], acc1[:, m], lane1[:])
                nc.vector.max(o1s[:, m], pk1[:])

            nc.sync.dma_start(
                out=o1_d.ap().rearrange("m p e -> p m e"), in_=o1s[:])
            nc.sync.dma_start(
                out=o2_d.ap().rearrange("c b p e -> p c b e"), in_=o2s[:])

    nc.compile()
    return nc


def _get_compiled():
    global _compiled
    if _compiled is None:
        _compiled = _build()
    return _compiled


def _normalize(fmap):
    d = fmap.reshape(CH, -1).astype(np.float32)
    nrm = np.sqrt(np.sum(np.square(d), axis=0, keepdims=True,
                         dtype=np.float32))
    return (d / nrm).astype(np.float32)


def _install_trace_shim():
    import types

    try:
        import antenv.axon_hooks  # noqa: F401
    except ImportError:
        from trn_agent_boot.trn_boot import _ntff_profile_via_ctypes
        hook = _ntff_profile_via_ctypes('/opt/axon/libaxon_pjrt.so')
        mod = types.ModuleType('antenv.axon_hooks')
        mod.get_axon_ntff_profile_hook = lambda: hook
        mod.set_axon_ntff_profile_hook = lambda h: None
        sys.modules['antenv.axon_hooks'] = mod
    import concourse.bass_utils as bu
    bu.upload_artifacts = lambda tmpdir: tmpdir


def _rerank(dq, dv, cand):
    """Exact top-2 per point. dq [CH, P] query descs, dv [CH, NV] value
    descs, cand [P, K] candidate indices (ascending per row). Returns
    (m1, i1, m2): best val, best idx (lowest on ties), second val."""
    P, K = cand.shape
    m1 = np.empty(P, np.float32)
    m2 = np.empty(P, np.float32)
    i1 = np.empty(P, np.int64)
    dqT = dq.T
    dvT = dv.T
    step = 2048
    for s in range(0, P, step):
        e = min(s + step, P)
        g = dvT[cand[s:e]]                       # [n, K, CH]
        sims = np.einsum('nkc,nc->nk', g, dqT[s:e],
                         optimize=True).astype(np.float32)
        j = np.argmax(sims, axis=1)
        r = np.arange(e - s)
        m1[s:e] = sims[r, j]
        i1[s:e] = cand[s:e][r, j]
        sims[r, j] = -np.inf
        m2[s:e] = sims.max(axis=1)
    return m1, i1, m2


def kernel(map_A, map_B):
    import os

    from concourse.bass_utils import run_bass_kernel_spmd
    from concourse import mybir

    global LAST_EXEC_NS, LAST_RESULTS
    trace = bool(int(os.environ.get("KERNEL_TRACE", "0")))
    if trace:
        _install_trace_shim()
    nc = _get_compiled()

    fp8 = np.dtype(mybir.dt.np(mybir.dt.float8e4))
    nA = _normalize(np.asarray(map_A))
    nB = _normalize(np.asarray(map_B))
    a8 = (nA * 32.0).astype(fp8)
    b8 = (nB * 32.0).astype(fp8)

    in_maps = []
    for c in range(N_CORES):
        sl = slice(c * SLAB, (c + 1) * SLAB)
        in_maps.append({
            "lhs8": np.ascontiguousarray(a8[:, sl]),
            "rhs8": b8,
        })

    res = run_bass_kernel_spmd(nc, in_maps, core_ids=list(range(N_CORES)),
                               trace=trace)
    LAST_EXEC_NS = res.exec_time_ns
    LAST_RESULTS = res

    # ---- direction 1: per-row top lanes -> candidate columns ----
    # o1 [MT, 128, 8] packed; row = core*SLAB + m*128 + p
    p1 = np.stack([res.results[c]["o1"] for c in range(N_CORES)])
    p1 = p1.transpose(0, 1, 2, 3).reshape(N_CORES, MT * 128, 8)
    p1 = p1.reshape(N1, 8)                        # sorted desc per row
    lanes1 = (p1.view(np.uint32) & 0x3FF).astype(np.int64)[:, :K1]
    cols = (lanes1[:, :, None]
            + (np.arange(NCB, dtype=np.int64) * CB)[None, None, :])
    cols = np.sort(cols.reshape(N1, K1 * NCB), axis=1)
    m1_12, nn12, m2_12 = _rerank(nA, nB, cols)

    # ---- direction 2: per-column top lane entries -> candidate rows ----
    # o2 [NCB, 8, 128, 8]; col = cb*CB + b*128 + p; lane = row in block
    p2 = np.stack([res.results[c]["o2"] for c in range(N_CORES)], axis=0)
    p2 = p2.transpose(1, 2, 3, 0, 4).reshape(N2, N_CORES, 8)
    lanes2 = (p2.view(np.uint32) & 0x3FF).astype(np.int64)
    base = (np.arange(N_CORES, dtype=np.int64) * SLAB)[None, :, None]
    rowbase = (lanes2 + base).reshape(N2, N_CORES * 8)
    vals2 = p2.reshape(N2, N_CORES * 8)
    order = np.argsort(-vals2, axis=1)[:, :K2]
    rb = np.take_along_axis(rowbase, order, axis=1)
    rows = (rb[:, :, None]
            + (np.arange(MT, dtype=np.int64) * 128)[None, None, :])
    rows = np.sort(rows.reshape(N2, K2 * MT), axis=1)
    m1_21, nn21, m2_21 = _rerank(nB, nA, rows)

    match_sim = m1_12
    ratios12 = (2.0 - 2.0 * m1_12) / ((2.0 - 2.0 * m2_12) + EPS)
    ratios21 = (2.0 - 2.0 * m1_21) / ((2.0 - 2.0 * m2_21) + EPS)

    ids1 = np.arange(N1)
    mask = ((ids1 == nn21[nn12]) & (ratios12 <= RATIO)
            & (ratios21[nn12] <= RATIO))
    masked_sim = np.where(mask, match_sim, 0.0).astype(np.float32)
    return masked_sim, nn12.astype(np.int32), mask


# revision 16
# speedup vs baseline: 4.2252x; 1.0899x over previous
"""v6: fp8 DoubleRow matmul + exp-sketch direction-2 + bf16 fold dir-1.

Each core computes its A-row slab sim [1536, 12288] with fp8e4m3
DoubleRow matmuls (inputs scaled x32 => PSUM holds 1024*sim). The
Activation engine converts each PSUM tile to q = exp(256*sim) in bf16
(one pass, monotone in sim; log-resolution 2^-9/256 ~ 8e-6).

Direction A->B: DVE folds q across the 12 column chunks into a
per-lane running max (tensor_max, 2x_1p), then one top-8 max +
max_index per row block.

Direction B->A: a *regular* matmul with a stationary bf16 identity
accumulates Sigma_m q[m] into fp32 PSUM (out = I.T @ q_m, start at
m=0): per column chunk this yields Sigma over the 12 row blocks of
exp(256*sim) per (row-lane, col) — a log-sum-exp sketch of the
per-lane max (inflation <= ln(12)/256 ~ 0.0097 in sim units, one
sided). The full [128, 12288] sketch is DMA'd out; the host ranks the
1024 (core, lane) values per column exactly.

The host expands each winning lane to its 12 fold members and re-ranks
all candidates with exact fp32 dots to get the true top-2 per
row/column, then applies the ratio and mutual-NN tests.
"""
import sys

sys.path.insert(0, '/opt/trn_rl_repo')

import numpy as np

CH = 512
N1 = 96 * 128
N2 = 96 * 128
N_CORES = 8
SLAB = N1 // N_CORES          # 1536
MT = SLAB // 128              # 12 row tiles per core
CB = 1024                     # column chunk
NCB = N2 // CB                # 12
RATIO = 0.95
EPS = 1e-8
K1 = 5                        # lanes kept per row (dir 1)
K2 = 10                       # lanes kept per column (dir 2)
BETA = 256.0

_compiled = None
LAST_EXEC_NS = None
LAST_RESULTS = None


def _build():
    import concourse.bacc as bacc
    import concourse.tile as tile
    from concourse import mybir

    nc = bacc.Bacc("TRN2", target_bir_lowering=False, debug=False,
                   num_devices=N_CORES)

    lhs_d = nc.dram_tensor("lhs8", [CH, SLAB], mybir.dt.float8e4,
                           kind="ExternalInput")
    rhs_d = nc.dram_tensor("rhs8", [CH, N2], mybir.dt.float8e4,
                           kind="ExternalInput")
    o1v_d = nc.dram_tensor("o1v", [MT, 128, 8], mybir.dt.bfloat16,
                           kind="ExternalOutput")
    o1i_d = nc.dram_tensor("o1i", [MT, 128, 8], mybir.dt.uint16,
                           kind="ExternalOutput")
    o2_d = nc.dram_tensor("o2", [NCB, 128, CB], mybir.dt.bfloat16,
                          kind="ExternalOutput")

    with tile.TileContext(nc) as tc:
        with tc.tile_pool(name="wts", bufs=1) as wts_pool, \
             tc.tile_pool(name="rhs", bufs=2) as rhs_pool, \
             tc.tile_pool(name="q", bufs=2) as q_pool, \
             tc.tile_pool(name="acc", bufs=1) as acc_pool, \
             tc.tile_pool(name="out", bufs=1) as out_pool, \
             tc.tile_pool(name="psmm", bufs=2, space="PSUM") as psmm_pool, \
             tc.tile_pool(name="ps2", bufs=2, space="PSUM") as ps2_pool:
            lh = wts_pool.tile([128, 4, SLAB], mybir.dt.float8e4, tag="lh")
            nc.sync.dma_start(
                out=lh[:],
                in_=lhs_d.ap().rearrange("(kt p) m -> p kt m", p=128))
            # bf16 identity for the accumulating pseudo-transpose
            import ml_dtypes
            identb_d = nc.inline_tensor(
                np.eye(128).astype(ml_dtypes.bfloat16), name="identb")
            identb = wts_pool.tile([128, 128], mybir.dt.bfloat16,
                                   tag="identb")
            nc.sync.dma_start(out=identb[:], in_=identb_d.ap())

            acc1 = acc_pool.tile([128, MT, CB], mybir.dt.bfloat16,
                                 tag="acc1")
            nc.gpsimd.memset(acc1[:], 0.0)
            o1v = out_pool.tile([128, MT, 8], mybir.dt.bfloat16, tag="o1v")
            o1i = out_pool.tile([128, MT, 8], mybir.dt.uint16, tag="o1i")

            for cb in range(NCB):
                rh = rhs_pool.tile([128, 4, CB], mybir.dt.float8e4, tag="rh")
                nc.sync.dma_start(
                    out=rh[:],
                    in_=rhs_d.ap()[:, cb * CB:(cb + 1) * CB]
                    .rearrange("(kt p) n -> p kt n", p=128))

                q2 = q_pool.tile([128, MT, CB], mybir.dt.bfloat16, tag="q2")
                for m in range(MT):
                    ps = psmm_pool.tile([128, CB], mybir.dt.float32,
                                        tag="ps")
                    msl = slice(m * 128, (m + 1) * 128)
                    for kp in range(2):
                        for c in range(CB // 512):
                            csl = slice(c * 512, (c + 1) * 512)
                            nc.tensor.matmul(
                                out=ps[:, csl],
                                lhsT=lh[:, 2 * kp:2 * kp + 2, msl],
                                rhs=rh[:, 2 * kp:2 * kp + 2, csl],
                                start=(kp == 0),
                                stop=(kp == 1),
                                perf_mode=mybir.MatmulPerfMode.DoubleRow)
                    # q = exp(256*sim) in bf16 (ps holds 1024*sim)
                    nc.scalar.activation(
                        out=q2[:, m], in_=ps[:],
                        func=mybir.ActivationFunctionType.Exp,
                        bias=0.0, scale=BETA / 1024.0)
                    if m % 2 == 1:
                        nc.vector.tensor_max(
                            acc1[:, m - 1:m + 1], acc1[:, m - 1:m + 1],
                            q2[:, m - 1:m + 1])

                # dir 2: acc2p[r, c] = Sigma_m q2[m][r, c] via identity
                # matmuls accumulating in PSUM
                acc2p = ps2_pool.tile([128, CB], mybir.dt.float32,
                                      tag="acc2p")
                for c in range(CB // 512):
                    csl = slice(c * 512, (c + 1) * 512)
                    for m in range(MT):
                        nc.tensor.matmul(
                            out=acc2p[:, csl],
                            lhsT=identb[:],
                            rhs=q2[:, m, csl],
                            start=(m == 0),
                            stop=(m == MT - 1))
                s2 = q_pool.tile([128, CB], mybir.dt.bfloat16, tag="s2")
                nc.vector.tensor_copy(s2[:], acc2p[:])
                nc.sync.dma_start(out=o2_d.ap()[cb], in_=s2[:])

            for m in range(MT):
                nc.vector.max(o1v[:, m], acc1[:, m])
                nc.vector.max_index(o1i[:, m], o1v[:, m], acc1[:, m])

            nc.sync.dma_start(
                out=o1v_d.ap().rearrange("m p e -> p m e"), in_=o1v[:])
            nc.sync.dma_start(
                out=o1i_d.ap().rearrange("m p e -> p m e"), in_=o1i[:])

    nc.compile()
    return nc


def _get_compiled():
    global _compiled
    if _compiled is None:
        _compiled = _build()
    return _compiled


def _normalize(fmap):
    d = fmap.reshape(CH, -1).astype(np.float32)
    nrm = np.sqrt(np.sum(np.square(d), axis=0, keepdims=True,
                         dtype=np.float32))
    return (d / nrm).astype(np.float32)


def _install_trace_shim():
    import types

    try:
        import antenv.axon_hooks  # noqa: F401
    except ImportError:
        from trn_agent_boot.trn_boot import _ntff_profile_via_ctypes
        hook = _ntff_profile_via_ctypes('/opt/axon/libaxon_pjrt.so')
        mod = types.ModuleType('antenv.axon_hooks')
        mod.get_axon_ntff_profile_hook = lambda: hook
        mod.set_axon_ntff_profile_hook = lambda h: None
        sys.modules['antenv.axon_hooks'] = mod
    import concourse.bass_utils as bu
    bu.upload_artifacts = lambda tmpdir: tmpdir


def _rerank(dq, dv, cand):
    """Exact top-2 per point. dq [CH, P] query descs, dv [CH, NV] value
    descs, cand [P, K] candidate indices (ascending per row). Returns
    (m1, i1, m2): best val, best idx (lowest on ties), second val."""
    P, K = cand.shape
    m1 = np.empty(P, np.float32)
    m2 = np.empty(P, np.float32)
    i1 = np.empty(P, np.int64)
    dqT = dq.T
    dvT = dv.T
    step = 1024
    for s in range(0, P, step):
        e = min(s + step, P)
        g = dvT[cand[s:e]]                       # [n, K, CH]
        sims = np.einsum('nkc,nc->nk', g, dqT[s:e],
                         optimize=True).astype(np.float32)
        j = np.argmax(sims, axis=1)
        r = np.arange(e - s)
        m1[s:e] = sims[r, j]
        i1[s:e] = cand[s:e][r, j]
        sims[r, j] = -np.inf
        m2[s:e] = sims.max(axis=1)
    return m1, i1, m2


def kernel(map_A, map_B):
    import os

    from concourse.bass_utils import run_bass_kernel_spmd
    from concourse import mybir

    global LAST_EXEC_NS, LAST_RESULTS
    trace = bool(int(os.environ.get("KERNEL_TRACE", "0")))
    if trace:
        _install_trace_shim()
    nc = _get_compiled()

    fp8 = np.dtype(mybir.dt.np(mybir.dt.float8e4))
    nA = _normalize(np.asarray(map_A))
    nB = _normalize(np.asarray(map_B))
    a8 = (nA * 32.0).astype(fp8)
    b8 = (nB * 32.0).astype(fp8)

    in_maps = []
    for c in range(N_CORES):
        sl = slice(c * SLAB, (c + 1) * SLAB)
        in_maps.append({
            "lhs8": np.ascontiguousarray(a8[:, sl]),
            "rhs8": b8,
        })

    res = run_bass_kernel_spmd(nc, in_maps, core_ids=list(range(N_CORES)),
                               trace=trace)
    LAST_EXEC_NS = res.exec_time_ns
    LAST_RESULTS = res

    # ---- direction 1: per-row top lanes -> candidate columns ----
    lanes1 = np.stack([res.results[c]["o1i"] for c in range(N_CORES)])
    lanes1 = lanes1.reshape(N1, 8).astype(np.int64)[:, :K1]
    cols = (lanes1[:, :, None]
            + (np.arange(NCB, dtype=np.int64) * CB)[None, None, :])
    cols = np.sort(cols.reshape(N1, K1 * NCB), axis=1)
    m1_12, nn12, m2_12 = _rerank(nA, nB, cols)

    # ---- direction 2: rank (core, lane) exp-sums per column ----
    # o2 [NCB, 128, CB] per core: Sigma_m exp(256 sim) at (lane r, col)
    s2 = np.stack([np.asarray(res.results[c]["o2"]).astype(np.float32)
                   for c in range(N_CORES)])
    # -> [col, core, r]
    s2 = s2.transpose(1, 3, 0, 2).reshape(N2, N_CORES * 128)
    top = np.argpartition(-s2, K2 - 1, axis=1)[:, :K2]
    core_id, lane = top // 128, top % 128
    rowbase = core_id * SLAB + lane
    rows = (rowbase[:, :, None]
            + (np.arange(MT, dtype=np.int64) * 128)[None, None, :])
    rows = np.sort(rows.reshape(N2, K2 * MT), axis=1)
    m1_21, nn21, m2_21 = _rerank(nB, nA, rows)

    match_sim = m1_12
    ratios12 = (2.0 - 2.0 * m1_12) / ((2.0 - 2.0 * m2_12) + EPS)
    ratios21 = (2.0 - 2.0 * m1_21) / ((2.0 - 2.0 * m2_21) + EPS)

    ids1 = np.arange(N1)
    mask = ((ids1 == nn21[nn12]) & (ratios12 <= RATIO)
            & (ratios21[nn12] <= RATIO))
    masked_sim = np.where(mask, match_sim, 0.0).astype(np.float32)
    return masked_sim, nn12.astype(np.int32), mask


# revision 19
# speedup vs baseline: 4.2471x; 1.0052x over previous
"""v6: fp8 DoubleRow matmul + exp-sketch direction-2 + bf16 fold dir-1.

Each core computes its A-row slab sim [1536, 12288] with fp8e4m3
DoubleRow matmuls (inputs scaled x32 => PSUM holds 1024*sim). The
Activation engine converts each PSUM tile to q = exp(256*sim) in bf16
(one pass, monotone in sim; log-resolution 2^-9/256 ~ 8e-6).

Direction A->B: DVE folds q across the 12 column chunks into a
per-lane running max (tensor_max, 2x_1p), then one top-8 max +
max_index per row block.

Direction B->A: a *regular* matmul with a stationary bf16 identity
accumulates Sigma_m q[m] into fp32 PSUM (out = I.T @ q_m, start at
m=0): per column chunk this yields Sigma over the 12 row blocks of
exp(256*sim) per (row-lane, col) — a log-sum-exp sketch of the
per-lane max (inflation <= ln(12)/256 ~ 0.0097 in sim units, one
sided). The full [128, 12288] sketch is DMA'd out; the host ranks the
1024 (core, lane) values per column exactly.

The host expands each winning lane to its 12 fold members and re-ranks
all candidates with exact fp32 dots to get the true top-2 per
row/column, then applies the ratio and mutual-NN tests.
"""
import sys

sys.path.insert(0, '/opt/trn_rl_repo')

import numpy as np

CH = 512
N1 = 96 * 128
N2 = 96 * 128
N_CORES = 8
SLAB = N1 // N_CORES          # 1536
MT = SLAB // 128              # 12 row tiles per core
CB = 1024                     # column chunk
NCB = N2 // CB                # 12
RATIO = 0.95
EPS = 1e-8
K1 = 5                        # lanes kept per row (dir 1)
K2 = 10                       # lanes kept per column (dir 2)
BETA = 256.0

_compiled = None
LAST_EXEC_NS = None
LAST_RESULTS = None


def _enable_ldw_opt():
    """Let walrus dedupe back-to-back LDWEIGHTS of identical weights
    (the accumulating identity matmuls reload the same 128x128 eye
    288 times otherwise)."""
    import concourse.bass_utils as bu

    if getattr(bu.run_command, "_ldw_opt", False):
        return
    orig = bu.run_command

    def run_command(cmd, *a, **kw):
        cmd = ["--enable-ldw-opt=true" if c == "--enable-ldw-opt=false"
               else c for c in cmd]
        return orig(cmd, *a, **kw)

    run_command._ldw_opt = True
    bu.run_command = run_command


def _build():
    import concourse.bacc as bacc
    import concourse.tile as tile
    from concourse import mybir

    nc = bacc.Bacc("TRN2", target_bir_lowering=False, debug=False,
                   num_devices=N_CORES)

    lhs_d = nc.dram_tensor("lhs8", [CH, SLAB], mybir.dt.float8e4,
                           kind="ExternalInput")
    rhs_d = nc.dram_tensor("rhs8", [CH, N2], mybir.dt.float8e4,
                           kind="ExternalInput")
    o1v_d = nc.dram_tensor("o1v", [MT, 128, 8], mybir.dt.bfloat16,
                           kind="ExternalOutput")
    o1i_d = nc.dram_tensor("o1i", [MT, 128, 8], mybir.dt.uint16,
                           kind="ExternalOutput")
    o2_d = nc.dram_tensor("o2", [NCB, 128, CB], mybir.dt.bfloat16,
                          kind="ExternalOutput")

    with tile.TileContext(nc) as tc:
        with tc.tile_pool(name="wts", bufs=1) as wts_pool, \
             tc.tile_pool(name="rhs", bufs=2) as rhs_pool, \
             tc.tile_pool(name="q", bufs=2) as q_pool, \
             tc.tile_pool(name="acc", bufs=1) as acc_pool, \
             tc.tile_pool(name="out", bufs=1) as out_pool, \
             tc.tile_pool(name="psmm", bufs=2, space="PSUM") as psmm_pool, \
             tc.tile_pool(name="ps2", bufs=2, space="PSUM") as ps2_pool:
            lh = wts_pool.tile([128, 4, SLAB], mybir.dt.float8e4, tag="lh")
            nc.sync.dma_start(
                out=lh[:],
                in_=lhs_d.ap().rearrange("(kt p) m -> p kt m", p=128))
            # bf16 identity for the accumulating pseudo-transpose
            import ml_dtypes
            identb_d = nc.inline_tensor(
                np.eye(128).astype(ml_dtypes.bfloat16), name="identb")
            identb = wts_pool.tile([128, 128], mybir.dt.bfloat16,
                                   tag="identb")
            nc.sync.dma_start(out=identb[:], in_=identb_d.ap())

            acc1 = acc_pool.tile([128, MT, CB], mybir.dt.bfloat16,
                                 tag="acc1")
            nc.gpsimd.memset(acc1[:], 0.0)
            o1v = out_pool.tile([128, MT, 8], mybir.dt.bfloat16, tag="o1v")
            o1i = out_pool.tile([128, MT, 8], mybir.dt.uint16, tag="o1i")

            for cb in range(NCB):
                rh = rhs_pool.tile([128, 4, CB], mybir.dt.float8e4, tag="rh")
                nc.sync.dma_start(
                    out=rh[:],
                    in_=rhs_d.ap()[:, cb * CB:(cb + 1) * CB]
                    .rearrange("(kt p) n -> p kt n", p=128))

                q2 = q_pool.tile([128, MT, CB], mybir.dt.bfloat16, tag="q2")
                for m in range(MT):
                    ps = psmm_pool.tile([128, CB], mybir.dt.float32,
                                        tag="ps")
                    msl = slice(m * 128, (m + 1) * 128)
                    for kp in range(2):
                        for c in range(CB // 512):
                            csl = slice(c * 512, (c + 1) * 512)
                            nc.tensor.matmul(
                                out=ps[:, csl],
                                lhsT=lh[:, 2 * kp:2 * kp + 2, msl],
                                rhs=rh[:, 2 * kp:2 * kp + 2, csl],
                                start=(kp == 0),
                                stop=(kp == 1),
                                perf_mode=mybir.MatmulPerfMode.DoubleRow)
                    # q = exp(256*sim) in bf16 (ps holds 1024*sim)
                    nc.scalar.activation(
                        out=q2[:, m], in_=ps[:],
                        func=mybir.ActivationFunctionType.Exp,
                        bias=0.0, scale=BETA / 1024.0)
                    if m % 2 == 1:
                        nc.vector.tensor_max(
                            acc1[:, m - 1:m + 1], acc1[:, m - 1:m + 1],
                            q2[:, m - 1:m + 1])
                    # dir 2: acc2p[r, c] += q2[m][r, c] via identity
                    # matmul accumulating in PSUM
                    if m == 0:
                        acc2p = ps2_pool.tile([128, CB], mybir.dt.float32,
                                              tag="acc2p")
                    for c in range(CB // 512):
                        csl = slice(c * 512, (c + 1) * 512)
                        nc.tensor.matmul(
                            out=acc2p[:, csl],
                            lhsT=identb[:],
                            rhs=q2[:, m, csl],
                            start=(m == 0),
                            stop=(m == MT - 1))

                s2 = q_pool.tile([128, CB], mybir.dt.bfloat16, tag="s2")
                nc.vector.tensor_copy(s2[:], acc2p[:])
                nc.sync.dma_start(out=o2_d.ap()[cb], in_=s2[:])

            for m in range(MT):
                nc.vector.max(o1v[:, m], acc1[:, m])
                nc.vector.max_index(o1i[:, m], o1v[:, m], acc1[:, m])

            nc.sync.dma_start(
                out=o1v_d.ap().rearrange("m p e -> p m e"), in_=o1v[:])
            nc.sync.dma_start(
                out=o1i_d.ap().rearrange("m p e -> p m e"), in_=o1i[:])

    nc.compile()
    return nc


def _get_compiled():
    global _compiled
    if _compiled is None:
        _compiled = _build()
    return _compiled


def _normalize(fmap):
    d = fmap.reshape(CH, -1).astype(np.float32)
    nrm = np.sqrt(np.sum(np.square(d), axis=0, keepdims=True,
                         dtype=np.float32))
    return (d / nrm).astype(np.float32)


def _install_trace_shim():
    import types

    try:
        import antenv.axon_hooks  # noqa: F401
    except ImportError:
        from trn_agent_boot.trn_boot import _ntff_profile_via_ctypes
        hook = _ntff_profile_via_ctypes('/opt/axon/libaxon_pjrt.so')
        mod = types.ModuleType('antenv.axon_hooks')
        mod.get_axon_ntff_profile_hook = lambda: hook
        mod.set_axon_ntff_profile_hook = lambda h: None
        sys.modules['antenv.axon_hooks'] = mod
    import concourse.bass_utils as bu
    bu.upload_artifacts = lambda tmpdir: tmpdir


def _rerank(dq, dv, cand):
    """Exact top-2 per point. dq [CH, P] query descs, dv [CH, NV] value
    descs, cand [P, K] candidate indices (ascending per row). Returns
    (m1, i1, m2): best val, best idx (lowest on ties), second val."""
    P, K = cand.shape
    m1 = np.empty(P, np.float32)
    m2 = np.empty(P, np.float32)
    i1 = np.empty(P, np.int64)
    dqT = dq.T
    dvT = dv.T
    step = 1024
    for s in range(0, P, step):
        e = min(s + step, P)
        g = dvT[cand[s:e]]                       # [n, K, CH]
        sims = np.einsum('nkc,nc->nk', g, dqT[s:e],
                         optimize=True).astype(np.float32)
        j = np.argmax(sims, axis=1)
        r = np.arange(e - s)
        m1[s:e] = sims[r, j]
        i1[s:e] = cand[s:e][r, j]
        sims[r, j] = -np.inf
        m2[s:e] = sims.max(axis=1)
    return m1, i1, m2


def kernel(map_A, map_B):
    import os

    from concourse.bass_utils import run_bass_kernel_spmd
    from concourse import mybir

    global LAST_EXEC_NS, LAST_RESULTS
    trace = bool(int(os.environ.get("KERNEL_TRACE", "0")))
    if trace:
        _install_trace_shim()
    nc = _get_compiled()

    fp8 = np.dtype(mybir.dt.np(mybir.dt.float8e4))
    nA = _normalize(np.asarray(map_A))
    nB = _normalize(np.asarray(map_B))
    a8 = (nA * 32.0).astype(fp8)
    b8 = (nB * 32.0).astype(fp8)

    in_maps = []
    for c in range(N_CORES):
        sl = slice(c * SLAB, (c + 1) * SLAB)
        in_maps.append({
            "lhs8": np.ascontiguousarray(a8[:, sl]),
            "rhs8": b8,
        })

    res = run_bass_kernel_spmd(nc, in_maps, core_ids=list(range(N_CORES)),
                               trace=trace)
    LAST_EXEC_NS = res.exec_time_ns
    LAST_RESULTS = res

    # ---- direction 1: per-row top lanes -> candidate columns ----
    lanes1 = np.stack([res.results[c]["o1i"] for c in range(N_CORES)])
    lanes1 = lanes1.reshape(N1, 8).astype(np.int64)[:, :K1]
    cols = (lanes1[:, :, None]
            + (np.arange(NCB, dtype=np.int64) * CB)[None, None, :])
    cols = np.sort(cols.reshape(N1, K1 * NCB), axis=1)
    m1_12, nn12, m2_12 = _rerank(nA, nB, cols)

    # ---- direction 2: rank (core, lane) exp-sums per column ----
    # o2 [NCB, 128, CB] per core: Sigma_m exp(256 sim) at (lane r, col)
    s2 = np.stack([np.asarray(res.results[c]["o2"]).astype(np.float32)
                   for c in range(N_CORES)])
    # -> [col, core, r]
    s2 = s2.transpose(1, 3, 0, 2).reshape(N2, N_CORES * 128)
    top = np.argpartition(-s2, K2 - 1, axis=1)[:, :K2]
    core_id, lane = top // 128, top % 128
    rowbase = core_id * SLAB + lane
    rows = (rowbase[:, :, None]
            + (np.arange(MT, dtype=np.int64) * 128)[None, None, :])
    rows = np.sort(rows.reshape(N2, K2 * MT), axis=1)
    m1_21, nn21, m2_21 = _rerank(nB, nA, rows)

    match_sim = m1_12
    ratios12 = (2.0 - 2.0 * m1_12) / ((2.0 - 2.0 * m2_12) + EPS)
    ratios21 = (2.0 - 2.0 * m1_21) / ((2.0 - 2.0 * m2_21) + EPS)

    ids1 = np.arange(N1)
    mask = ((ids1 == nn21[nn12]) & (ratios12 <= RATIO)
            & (ratios21[nn12] <= RATIO))
    masked_sim = np.where(mask, match_sim, 0.0).astype(np.float32)
    return masked_sim, nn12.astype(np.int32), mask
